# revision 9
# baseline (speedup 1.0000x reference)
"""Distributed multi-head attention kernel for 8 TRN2 NeuronCores.

Reference computation (per batch b):
    q = x @ wq.T ; k = x @ wk.T ; v = x @ wv.T          (heads split from 512 -> 8 x 64)
    attn = softmax(q k^T / sqrt(64)) ; o = attn @ v
    y = concat_heads(o) @ wproj.T

Sharding: core c handles batch b = c // 4 and head-block hb = c % 4
(2 heads = 128 channels).  Within a 4-core replica group (one batch) the
normalized head outputs are AllGather'ed (chunked along the query axis,
overlapped with attention compute) and each core computes a column block
(128 output channels) of the final projection.

Engine balance: softmax exp is the per-core floor (2*N^2 elements).  It
is split between the Scalar engine (true exp) and a custom DVE op
(EXP2_BF16_ANT: Schraudolph-style exp2 with quadratic mantissa
correction, ~0.3% rms) so the Tensor engine becomes the only bottleneck.
wq is pre-scaled by SCALE*128/ln2 so score PSUM is already in
128*log2-domain: the DVE op consumes it directly, the Scalar exp
applies scale=ln2/128.  Normalization (reciprocal of the appended
ones-row output, partition-broadcast, multiply) runs on DVE
(reciprocal_approx_fast) + GpSimd; PSUM->SBUF evictions run on GpSimd.
"""

import sys

sys.path.insert(0, "/opt/trn_rl_repo")

import math

import ml_dtypes
import numpy as np

B = 2
N = 3136
DIM = 512
HEADS = 8
HD = 64
SCALE = HD**-0.5
N_CORES = 8
GROUPS = [[0, 1, 2, 3], [4, 5, 6, 7]]

BF16 = ml_dtypes.bfloat16

DVE_EXP = False  # route a share of softmax exps to the custom DVE op

# custom exp op constants (fit in fit_exp; verified on HW: max 0.71% rel)
EXP_M = 3.0 * 2.0**29
EXP_B64 = 16309.689999999999
EXP_GAMMA = 0.0024151
LOG2E_128 = 128.0 / math.log(2.0)

# query/row chunks of 512 (last 64), key tiles of 128 (last 64)
QCH = [(o, min(512, N - o)) for o in range(0, N, 512)]
MT = [(o, min(128, N - o)) for o in range(0, N, 128)]
# AllGather parts: after query-chunk qi, gather columns [off, off+len)
AG_AFTER = {i: (o, n) for i, (o, n) in enumerate(QCH)}

_CACHE = {}


def _register_exp_op():
    import concourse.dve_ops as dve_ops
    from concourse.dve_ops import DveOp
    from concourse.dve_spec import (
        Spec,
        Src0,
        C0,
        C1,
        C2,
        C3,
        lower,
        _spill_c3_to_src1,
        _has_src1,
    )
    from concourse.dve_uop import DveOpSpec

    name = "EXP2_BF16_ANT"
    if name in dve_ops._SUB_OPCODE_FOR_NAME:
        return next(o for o in dve_ops.OPS if o.name == name)

    # int16 out = (y - 64 + B64) + gamma*f0^2 with f0 = (y mod 128) - 64;
    # bitcast(int16) == exp(y*ln2/128) to ~0.34% rms (global scale == 1).
    z = Src0 + C2  # imm2 = -64
    A = z + C0  # s0 = M (magic round-to-128)
    bP = A - C0
    f0 = z - bP
    w = z + C1  # s1 = B64
    t = f0 * C3  # gamma via Latch(Src1)
    g = f0 * t
    v = w + g
    body = _spill_c3_to_src1(v)

    def ref(in0, in1, s0, s1, imm2):
        y = in0.astype(np.float32)
        z = (y + np.float32(imm2)).astype(np.float32)
        A = (z + np.float32(s0)).astype(np.float32)
        bP = (A - np.float32(s0)).astype(np.float32)
        f0 = (z - bP).astype(np.float32)
        return ((z + np.float32(s1)) + f0 * (f0 * in1)).astype(np.float32)

    spec = Spec(body=body, reference=ref)
    row = max(dve_ops._SUB_OPCODE_FOR_NAME.values()) + 1
    assert row < 0x20
    uops = lower(spec, ver="v3")
    sha = DveOpSpec(name=name, opcode=row, uops=uops, rd1_en=_has_src1(spec)).sha("v3")
    op = DveOp(name, spec, subdim=False, uops_sha={"v3": sha})
    dve_ops.OPS.append(op)
    dve_ops.CUSTOM_DVE_SPECS[name] = op.spec
    dve_ops._SUB_OPCODE_FOR_NAME[name] = row
    return op


def _build(debug_dumps=False):
    import concourse.bacc as bacc
    import concourse.mybir as mybir
    import concourse.tile as tile
    from concourse.bass_interp import get_hw_module

    EXP_OP = _register_exp_op()

    F32 = mybir.dt.float32
    BF = mybir.dt.bfloat16
    I16 = mybir.dt.int16

    nc = bacc.Bacc("TRN2", target_bir_lowering=False, debug=False, num_devices=N_CORES)

    xT_d = nc.dram_tensor("xT", [DIM, N], BF, kind="ExternalInput")
    wq_d = nc.dram_tensor("wqT", [DIM, 128], BF, kind="ExternalInput")
    wk_d = nc.dram_tensor("wkT", [DIM, 128], BF, kind="ExternalInput")
    wv_d = nc.dram_tensor("wvT", [DIM, 128], BF, kind="ExternalInput")
    wp_d = nc.dram_tensor("wpT", [DIM, 128], BF, kind="ExternalInput")
    out_d = nc.dram_tensor("out", [128, N], F32, kind="ExternalOutput")

    EXP = mybir.ActivationFunctionType.Exp
    LN2_128 = float(math.log(2.0) / 128.0)

    with tile.TileContext(nc) as tc:
        with (
            tc.tile_pool(name="const", bufs=1) as cp,
            tc.tile_pool(name="big", bufs=1) as bp,
            tc.tile_pool(name="attn", bufs=4) as ap_,
            tc.tile_pool(name="norm", bufs=3) as np_,
            tc.tile_pool(name="gat", bufs=2) as gp,
            tc.tile_pool(name="psum", bufs=2, space="PSUM") as pa,
            tc.tile_pool(name="dram", bufs=1, space="DRAM") as dram,
        ):
            # ---- tiny warmup collective: absorbs collective-subsystem init
            # concurrently with the compute prologue ----
            wtiny = cp.tile([1, 16], BF)
            nc.vector.memset(wtiny[:], 0.0)
            wi = dram.tile([1, 16], BF)
            wo = dram.tile([4, 16], BF)
            nc.gpsimd.dma_start(wi[:], wtiny[:])
            nc.gpsimd.collective_compute(
                "AllGather",
                mybir.AluOpType.bypass,
                replica_groups=GROUPS,
                ins=[wi.opt()],
                outs=[wo.opt()],
            )

            gam = cp.tile([128, 1], F32)
            nc.vector.memset(gam[:], EXP_GAMMA)

            # ---- load inputs (weights first: they gate the first matmuls) ----
            wqT = cp.tile([128, 4, 128], BF)
            wkT = cp.tile([128, 4, 128], BF)
            wvT = cp.tile([128, 4, 128], BF)
            wpT = cp.tile([128, 4, 128], BF)
            for t, d in ((wkT, wk_d), (wqT, wq_d), (wvT, wv_d), (wpT, wp_d)):
                for k in range(4):
                    nc.gpsimd.dma_start(t[:, k, :], d[128 * k : 128 * (k + 1), :])
            xT = bp.tile([128, 4, N], BF)  # xT[:, k, :] = channels [128k,128k+128)
            for lo, hi in ((0, 512), (512, 1024), (1024, 2048), (2048, N)):
                for k in range(4):
                    nc.sync.dma_start(
                        xT[:, k, lo:hi], xT_d[128 * k : 128 * (k + 1), lo:hi]
                    )

            # ---- qkv projections ----
            qT = bp.tile([128, N], BF)  # rows 0-63 head0, 64-127 head1
            kT = bp.tile([128, N], BF)
            v1 = bp.tile([128, len(MT), 2, HD + 1], BF)  # [key, mtile, head, hd|1]
            nc.vector.memset(v1[:, :, :, HD : HD + 1], 1.0)

            def produce_chunk(wt, dst, qo, qn):
                ps = pa.tile([128, 2, 512], F32, tag="pp", name="ps")
                for k in range(4):
                    nc.tensor.matmul(
                        ps[:, 0, :qn],
                        wt[:, k, :],
                        xT[:, k, qo : qo + qn],
                        start=(k == 0),
                        stop=(k == 3),
                    )
                nc.vector.tensor_copy(dst[:, qo : qo + qn], ps[:, 0, :qn])

            def produce_v1(mi):
                mo, mn = MT[mi]
                ps = pa.tile([128, 2, 512], F32, tag="pp", name="ps")
                for k in range(4):
                    nc.tensor.matmul(
                        ps[:mn, 0, :128],
                        xT[:, k, mo : mo + mn],
                        wvT[:, k, :],
                        start=(k == 0),
                        stop=(k == 3),
                    )
                nc.vector.tensor_copy(v1[:mn, mi, 0, 0:HD], ps[:mn, 0, 0:HD])
                nc.vector.tensor_copy(v1[:mn, mi, 1, 0:HD], ps[:mn, 0, HD:128])

            # prologue: only what attention chunk 0 needs up front; the rest
            # of kT / qT / v1 is produced just-in-time inside the first two
            # attention chunks so the exp engines start early.
            produce_chunk(wkT, kT, *QCH[0])
            produce_chunk(wqT, qT, *QCH[0])
            for mi in range(2):
                produce_v1(mi)

            def emit_exp(dst_i16, pp, pmn, hs, cols, to_dve):
                """exp of pp[:pmn, hs, cols] -> dst tile (bf16 bit pattern)."""
                if to_dve and DVE_EXP:
                    nc.vector._custom_dve(
                        EXP_OP,
                        out=dst_i16[:pmn, hs, cols],
                        in0=pp[:pmn, hs, cols],
                        in1=gam[:pmn, :],
                        s0=EXP_M,
                        s1=EXP_B64,
                        imm2=-64.0,
                    )
                else:
                    nc.scalar.activation(
                        dst_i16[:pmn, hs, cols].bitcast(BF),
                        pp[:pmn, hs, cols],
                        EXP,
                        scale=LN2_128,
                    )

            # ---- attention ----
            outn = [bp.tile([64, N], BF, name=f"outn{h}") for h in range(2)]
            ag_bufs = []  # (ago, part_offset, part_len) per part, for projection
            exp_tick = 0  # round-robin splitter between Scalar and DVE

            for qi, (qo, qn) in enumerate(QCH):
                po = pa.tile([128, 2, 512], F32, tag="po")
                if qn == 512:
                    for mi, (mo, mn) in enumerate(MT):
                        pp = pa.tile([128, 2, 512], F32, tag="pp")
                        at = ap_.tile([128, 2, 512], I16, tag="at")
                        for h in range(2):
                            hs = slice(64 * h, 64 * (h + 1))
                            nc.tensor.matmul(
                                pp[:mn, h, :qn],
                                kT[hs, mo : mo + mn],
                                qT[hs, qo : qo + qn],
                                start=True,
                                stop=True,
                            )
                        # split exp between Scalar (5/9) and DVE (4/9)
                        exp_tick += 1
                        to_dve = exp_tick % 9 in (1, 3, 5, 7)
                        emit_exp(at, pp, mn, slice(0, 2), slice(0, qn), to_dve)
                        for h in range(2):
                            nc.tensor.matmul(
                                po[0:65, h, :qn],
                                v1[:mn, mi, h, :],
                                at[:mn, h, :qn].bitcast(BF),
                                start=(mi == 0),
                                stop=(mi == len(MT) - 1),
                            )
                        if qi == 0:
                            # JIT production: kT chunk j gates mi >= 4j;
                            # v1 tile mi+2 gates AV two tiles ahead.
                            if mi + 2 < len(MT):
                                produce_v1(mi + 2)
                            if mi % 4 == 1 and 1 + mi // 4 < len(QCH):
                                produce_chunk(wkT, kT, *QCH[1 + mi // 4])
                            if mi == 1:
                                produce_chunk(wqT, qT, *QCH[1])
                        elif 1 <= qi <= 4 and mi in (8, 17):
                            j = 2 + (qi - 1) * 2 + (mi - 8) // 9
                            if j < len(QCH) and j > qi:
                                produce_chunk(wqT, qT, *QCH[j])
                else:
                    # ragged 64-query tail: pack 8 key tiles x 2 heads per
                    # psum tile (8 slots of 64 per bank) so exp stays
                    # amortized at N=1024
                    for g0 in range(0, len(MT), 8):
                        ms = list(enumerate(MT))[g0 : g0 + 8]
                        pp = pa.tile([128, 2, 512], F32, tag="pp")
                        at = ap_.tile([128, 2, 512], I16, tag="at")
                        for s, (mi, (mo, mn)) in enumerate(ms):
                            for h in range(2):
                                hs = slice(64 * h, 64 * (h + 1))
                                nc.tensor.matmul(
                                    pp[:mn, h, 64 * s : 64 * s + qn],
                                    kT[hs, mo : mo + mn],
                                    qT[hs, qo : qo + qn],
                                    start=True,
                                    stop=True,
                                )
                        pmn = max(mn for _, (_, mn) in ms)
                        exp_tick += 1
                        emit_exp(
                            at,
                            pp,
                            pmn,
                            slice(0, 2),
                            slice(0, 64 * len(ms)),
                            exp_tick % 2 == 0,
                        )
                        for s, (mi, (mo, mn)) in enumerate(ms):
                            for h in range(2):
                                nc.tensor.matmul(
                                    po[0:65, h, :qn],
                                    v1[:mn, mi, h, :],
                                    at[:mn, h, 64 * s : 64 * s + qn].bitcast(BF),
                                    start=(mi == 0),
                                    stop=(mi == len(MT) - 1),
                                )
                # normalize rows 0-63 by row 64 (softmax denominator).
                # NB: partition_broadcast mis-reads APs whose base
                # partition != 0 on HW, so land the reciprocal on p0.
                rss, rbs = [], []
                for h in range(2):
                    # custom-DVE ops mis-read nonzero base partitions: stage the
                    # denominator row onto partition 0 first (standard copy).
                    dn = np_.tile([1, 512], F32, tag="dn", name=f"dn{h}")
                    rs = np_.tile([1, 512], F32, tag="rs", name=f"rs{h}")
                    nc.vector.tensor_copy(dn[0:1, :qn], po[64:65, h, :qn])
                    nc.vector.reciprocal_approx_fast(rs[0:1, :qn], dn[0:1, :qn])
                    rss.append(rs)
                for h in range(2):
                    rb = np_.tile([64, 512], F32, tag="rb", name=f"rb{h}")
                    nc.gpsimd.partition_broadcast(rb[:, :qn], rss[h][0:1, :qn])
                    rbs.append(rb)
                for h in range(2):
                    nc.vector.tensor_mul(
                        outn[h][:, qo : qo + qn], po[0:64, h, :qn], rbs[h][:, :qn]
                    )

                if qi not in AG_AFTER:
                    continue
                # ---- AllGather this part (overlaps remaining attention) ----
                pof, pln = AG_AFTER[qi]
                pi = list(AG_AFTER).index(qi)
                agi = dram.tile([128, pln], BF, name=f"agi{pi}")
                ago = dram.tile([DIM, pln], BF, name=f"ago{pi}")
                for h in range(2):
                    nc.sync.dma_start(
                        agi[64 * h : 64 * (h + 1), :], outn[h][:, pof : pof + pln]
                    )
                nc.gpsimd.collective_compute(
                    "AllGather",
                    mybir.AluOpType.bypass,
                    replica_groups=GROUPS,
                    ins=[agi.opt()],
                    outs=[ago.opt()],
                )
                ag_bufs.append((ago, pof, pln))

            if debug_dumps:
                dbg = {
                    name: nc.dram_tensor(name, shape, BF, kind="ExternalOutput")
                    for name, shape in (
                        ("dbg_qT", [128, N]),
                        ("dbg_kT", [128, N]),
                        ("dbg_outn0", [64, N]),
                        ("dbg_outn1", [64, N]),
                    )
                }
                nc.sync.dma_start(dbg["dbg_qT"][:], qT[:])
                nc.sync.dma_start(dbg["dbg_kT"][:], kT[:])
                nc.sync.dma_start(dbg["dbg_outn0"][:], outn[0][:])
                nc.sync.dma_start(dbg["dbg_outn1"][:], outn[1][:])

            # ---- projection (column-parallel: this core's 128 out-channels) ----
            yt = bp.tile([128, N], F32)
            for ago, pof, pln in ag_bufs:
                g = gp.tile([128, 4, 512], BF, tag="g")
                for k in range(4):
                    nc.sync.dma_start(g[:, k, :pln], ago[128 * k : 128 * (k + 1), :])
                py = pa.tile([128, 2, 512], F32, tag="pp")
                for k in range(4):
                    nc.tensor.matmul(
                        py[:, 0, :pln],
                        wpT[:, k, :],
                        g[:, k, :pln],
                        start=(k == 0),
                        stop=(k == 3),
                    )
                nc.vector.tensor_copy(yt[:, pof : pof + pln], py[:, 0, :pln])
                nc.sync.dma_start(out_d[:, pof : pof + pln], yt[:, pof : pof + pln])

    nc.compile()
    nc.m = get_hw_module(nc.m)
    return nc


def _shard(x, wq, wk, wv, wproj):
    x = np.asarray(x, dtype=np.float32)
    wq = np.asarray(wq, dtype=np.float32)
    wk = np.asarray(wk, dtype=np.float32)
    wv = np.asarray(wv, dtype=np.float32)
    wproj = np.asarray(wproj, dtype=np.float32)

    xT = [np.ascontiguousarray(x[b].T).astype(BF16) for b in range(B)]
    qscale = SCALE * LOG2E_128
    in_maps = []
    for c in range(N_CORES):
        b, hb = c // 4, c % 4
        rows = slice(128 * hb, 128 * (hb + 1))
        in_maps.append(
            {
                "xT": xT[b],
                "wqT": np.ascontiguousarray((wq[rows] * qscale).T).astype(BF16),
                "wkT": np.ascontiguousarray(wk[rows].T).astype(BF16),
                "wvT": np.ascontiguousarray(wv[rows].T).astype(BF16),
                "wpT": np.ascontiguousarray(wproj[rows].T).astype(BF16),
            }
        )
    return in_maps


def _run(inputs, trace=False):
    from concourse.bass_utils import run_bass_kernel_spmd

    if "nc" not in _CACHE:
        _CACHE["nc"] = _build()
    nc = _CACHE["nc"]
    in_maps = _shard(**inputs)
    res = run_bass_kernel_spmd(
        nc, in_maps, core_ids=list(range(N_CORES)), trace=trace
    )
    out = np.empty((B, N, DIM), dtype=np.float32)
    for c in range(N_CORES):
        b, hb = c // 4, c % 4
        out[b, :, 128 * hb : 128 * (hb + 1)] = res.results[c]["out"].T
    return out, res.exec_time_ns


def kernel(**inputs) -> np.ndarray:
    return _run(inputs, trace=False)[0]
